# revision 35
# baseline (speedup 1.0000x reference)
"""Trainium2 Bass kernel for nn_CrossModalAttention.

Problem: bidirectional cross-attention between two (B, C, H, W) feature maps.
  B=4, C=256, H=W=64 -> N=4096 pixels, HID=64.
  For each direction:  q = Wq@xq, k = Wk@xkv, v = Wv@xkv (1x1 convs),
  attn = softmax_m(q^T k), out = xq + gamma * (v @ attn^T).

Sharding: 2 directions x 4 batches = 8 independent units, one per NeuronCore.

Per-core kernel layout trick: compute S^T tiles [m(part)=128, n(free)=512] via
matmul(lhsT=k_tile, rhs=q_tile) (contraction over HID=64 on partitions), exp on
ScalarE (logits are bounded ~ +-56, so exp in f32 needs no max-subtraction),
then accumulate U[c, n] = sum_m vT[m, c]^T expS^T[m, n] directly in PSUM across
the 32 m-blocks -- no transposes anywhere. Denominator d[n] = sum_m expS^T via
a ones[128,1] matmul accumulated in PSUM. Final: out = xq + (gamma/d)*U + gamma*bv.

Precision: S path (projections + S matmul) in float32r (TF32-like; moving dim
512 >= 256 runs at full PE rate), expS^T/vT in bf16 (measured to contribute
negligible error), all accumulation in f32 PSUM.
"""

import sys

if "/opt/trn_rl_repo" not in sys.path:
    sys.path.insert(0, "/opt/trn_rl_repo")

import ml_dtypes
import numpy as np

B = 4
C = 256
HID = 64
N = 4096          # H*W
P = 128           # SBUF partitions
NT = 512          # n-tile (matmul moving free dim)
N_NT = N // NT    # 8
MB = 128          # m-block (PV contraction tile)
N_MB = N // MB    # 32
CA = C // P       # 2 c-chunks / c-blocks

_CACHE = {}


def _build_program():
    import concourse.bass as bass
    import concourse.mybir as mybir
    from concourse import tile

    f32 = mybir.dt.float32
    f32r = mybir.dt.float32r
    bf16 = mybir.dt.bfloat16
    AF = mybir.ActivationFunctionType

    nc = bass.Bass("TRN2", target_bir_lowering=False, debug=False)

    xq_d = nc.dram_tensor("xq", (C, N), f32r, kind="ExternalInput")
    xkv_d = nc.dram_tensor("xkv", (C, N), f32r, kind="ExternalInput")
    wqT_d = nc.dram_tensor("wqT", (C, HID), f32r, kind="ExternalInput")
    wkT_d = nc.dram_tensor("wkT", (C, HID), f32r, kind="ExternalInput")
    wvT_d = nc.dram_tensor("wvT", (C, C), f32r, kind="ExternalInput")
    bq_d = nc.dram_tensor("bq", (HID, 1), f32, kind="ExternalInput")
    bk_d = nc.dram_tensor("bk", (HID, 1), f32, kind="ExternalInput")
    gbv_d = nc.dram_tensor("gbv", (C, 1), f32, kind="ExternalInput")      # gamma * bv
    rgam_d = nc.dram_tensor("rgam", (1, 1), f32, kind="ExternalInput")    # 1 / gamma
    onesr_d = nc.dram_tensor("onesr", (1, P), f32r, kind="ExternalInput")  # all-ones
    xkvb_d = nc.dram_tensor("xkvb", (C, N), bf16, kind="ExternalInput")    # bf16 xkv
    out_d = nc.dram_tensor("out", (C, N), f32, kind="ExternalOutput")

    # c = a*128 + p views
    xq_r = xq_d[:].rearrange("(a p) n -> p a n", p=P)
    xkv_r = xkv_d[:].rearrange("(a p) n -> p a n", p=P)
    wqT_r = wqT_d[:].rearrange("(a p) h -> p a h", p=P)
    wkT_r = wkT_d[:].rearrange("(a p) h -> p a h", p=P)
    wvT_r = wvT_d[:].rearrange("(a p) c -> p a c", p=P)
    gbv_r = gbv_d[:].rearrange("(a p) one -> p (a one)", p=P)
    out_r = out_d[:].rearrange("(a p) n -> p a n", p=P)

    with tile.TileContext(nc) as tc:
        with (
            tc.tile_pool(name="const", bufs=1) as const,
            tc.tile_pool(name="xin", bufs=1) as xin,
            tc.tile_pool(name="qk", bufs=1) as qk,
            tc.tile_pool(name="vtp", bufs=1) as vtp,
            tc.tile_pool(name="work", bufs=3) as work,
            tc.tile_pool(name="ep", bufs=2) as ep,
            tc.tile_pool(name="dram", bufs=2, space="DRAM") as dram,
            tc.tile_pool(name="psum", bufs=1, space="PSUM") as psum,
        ):
            # ---- constants / weights ----
            wq_sb = const.tile([P, CA, HID], f32r, tag="wq")
            nc.sync.dma_start(wq_sb[:], wqT_r)
            wk_sb = const.tile([P, CA, HID], f32r, tag="wk")
            nc.sync.dma_start(wk_sb[:], wkT_r)
            wv_sb = const.tile([P, CA, C], f32r, tag="wv")
            nc.sync.dma_start(wv_sb[:], wvT_r)
            bq_sb = const.tile([HID, 1], f32, tag="bq")
            nc.sync.dma_start(bq_sb[:], bq_d[:])
            bk_sb = const.tile([HID, 1], f32, tag="bk")
            nc.sync.dma_start(bk_sb[:], bk_d[:])
            gbv_sb = const.tile([P, CA], f32, tag="gbv")
            nc.sync.dma_start(gbv_sb[:], gbv_r)
            rgam_sb = const.tile([1, 1], f32, tag="rgam")
            nc.sync.dma_start(rgam_sb[:], rgam_d[:])
            ones_sb = const.tile([P, 1], bf16, tag="ones")
            nc.vector.memset(ones_sb[:], 1.0)
            onesr_sb = const.tile([1, P], f32r, tag="onesr")
            nc.sync.dma_start(onesr_sb[:], onesr_d[:])

            # ---- x loads (chunked for DMA/compute overlap) ----
            xq_sb = xin.tile([P, CA, N], f32r, tag="xq")
            xkv_sb = xin.tile([P, CA, N], f32r, tag="xkv")
            NCH = 1024
            for h in range(N // NCH):
                sl = slice(h * NCH, (h + 1) * NCH)
                for a in range(CA):
                    nc.sync.dma_start(xkv_sb[:, a, sl], xkv_r[:, a, sl])
            for h in range(N // NCH):
                sl = slice(h * NCH, (h + 1) * NCH)
                for a in range(CA):
                    nc.sync.dma_start(xq_sb[:, a, sl], xq_r[:, a, sl])

            # ---- projections ----
            # q/k stored twice (rows 0-63 and 64-127) so the K=64 S-matmuls
            # can be row-paired into both halves of the PE array.
            q_sb = qk.tile([P, N], f32r, tag="q")
            k_sb = qk.tile([P, N], f32r, tag="k")
            # k projection first: it only needs xkv, which is DMA'd first
            for nt in range(N_NT):
                ntsl = slice(nt * NT, (nt + 1) * NT)
                kp = psum.tile([P, NT], f32, tag="st", bufs=3)
                for a in range(CA):
                    nc.tensor.matmul(
                        kp[:HID, :],
                        lhsT=wk_sb[:, a, :],
                        rhs=xkv_sb[:, a, ntsl],
                        start=(a == 0),
                        stop=(a == CA - 1),
                    )
                nc.vector.tensor_scalar_add(k_sb[0:HID, ntsl], kp[:HID, :], bk_sb[:])
                nc.vector.tensor_scalar_add(k_sb[HID:P, ntsl], kp[:HID, :], bk_sb[:])

            # X2^T tiles [m, c_in] (bf16) for the Y = X2 @ E matmuls, loaded
            # with DMA-transpose from the host-cast bf16 copy of xkv. The v
            # projection itself is folded in AFTER the attention sum via
            # associativity: U = Wv @ (X2 @ E) -- saves the whole vT pass.
            x2t_sb = vtp.tile([P, N_MB, C], bf16, tag="x2t")
            for mb in range(N_MB):
                nc.scalar.dma_start(
                    x2t_sb[:, mb, :],
                    xkvb_d[:][:, mb * MB : (mb + 1) * MB],
                    transpose=True,
                )

            # q projection (needs xq, which lands after xkv)
            for nt in range(N_NT):
                ntsl = slice(nt * NT, (nt + 1) * NT)
                qp = psum.tile([P, NT], f32, tag="st", bufs=3)
                for a in range(CA):
                    nc.tensor.matmul(
                        qp[:HID, :],
                        lhsT=wq_sb[:, a, :],
                        rhs=xq_sb[:, a, ntsl],
                        start=(a == 0),
                        stop=(a == CA - 1),
                    )
                nc.vector.tensor_scalar_add(q_sb[0:HID, ntsl], qp[:HID, :], bq_sb[:])
                nc.vector.tensor_scalar_add(q_sb[HID:P, ntsl], qp[:HID, :], bq_sb[:])

            # ---- attention ----
            DG = 8           # m-blocks per denominator group
            N_DG = N_MB // DG

            def _epilogue_a(nt, y0, y1, dp, final_dmm):
                # finish d, grd = gamma/d on ACT+DVE, kick off the partition-
                # broadcast DRAM roundtrip, and move Y to SBUF (no PE work)
                final_dmm()
                rd = ep.tile([1, NT], f32, tag="rd", name=f"rd_{nt}")
                nc.scalar.activation(rd[:], dp[:], AF.Copy, scale=rgam_sb[:])
                grd = ep.tile([1, NT], f32, tag="grd", name=f"grd_{nt}")
                nc.vector.reciprocal(grd[:], rd[:])
                dscr = dram.tile([1, NT], f32, tag="dscr", name=f"dscr_{nt}")
                nc.sync.dma_start(dscr[:], grd[:])
                rdb = ep.tile([P, NT], f32, tag="rdb", name=f"rdb_{nt}")
                nc.sync.dma_start(rdb[:], dscr[:].broadcast_to((P, NT)))
                yb0 = ep.tile([P, NT], f32r, tag="yb0", name=f"yb0_{nt}")
                nc.scalar.copy(yb0[:], y0[:])
                yb1 = ep.tile([P, NT], f32r, tag="yb1", name=f"yb1_{nt}")
                nc.scalar.copy(yb1[:], y1[:])
                return rdb, yb0, yb1

            def _epilogue_b(nt, state):
                # U = Wv @ Y (f32r, 2 accumulating matmuls per c-block), then
                # out[c, n] = xq + rdb[n] * U[c, n] + gamma*bv[c]
                rdb, yb0, yb1 = state
                ntsl = slice(nt * NT, (nt + 1) * NT)
                for cb in range(CA):
                    ups = psum.tile(
                        [P, NT], f32, tag="st", bufs=3, name=f"ups_{nt}_{cb}"
                    )
                    nc.tensor.matmul(
                        ups[:], lhsT=wv_sb[:, 0, cb * P : (cb + 1) * P],
                        rhs=yb0[:], start=True, stop=False,
                    )
                    nc.tensor.matmul(
                        ups[:], lhsT=wv_sb[:, 1, cb * P : (cb + 1) * P],
                        rhs=yb1[:], start=False, stop=True,
                    )
                    t = ep.tile([P, NT], f32, tag="t", name=f"t_{nt}_{cb}")
                    nc.vector.tensor_mul(t[:], ups[:], rdb[:])
                    o = ep.tile([P, NT], f32, tag="o", name=f"o_{nt}_{cb}")
                    nc.vector.scalar_tensor_tensor(
                        o[:],
                        in0=t[:],
                        scalar=gbv_sb[:, cb : cb + 1],
                        in1=xq_sb[:, cb, ntsl],
                        op0=mybir.AluOpType.add,
                        op1=mybir.AluOpType.add,
                    )
                    nc.sync.dma_start(out_r[:, cb, ntsl], o[:])

            # previous n-tile's epilogue stages, deferred into the next
            # n-tile's m-loop so they never stall the in-order PE queue
            pending_a = [None]
            pending_b = [None]

            for nt in range(N_NT):
                ntsl = slice(nt * NT, (nt + 1) * NT)
                y0 = psum.tile([P, NT], f32, tag="y", bufs=4, name=f"y0_{nt}")
                y1 = psum.tile([P, NT], f32, tag="y", bufs=4, name=f"y1_{nt}")
                dp = psum.tile([1, NT], f32, tag="dd", bufs=1, name=f"dp_{nt}")
                acc = None   # running bf16 partial-sum for the current d group
                n_d = 0      # d-matmuls issued for this n-tile
                for mb in range(N_MB):
                    msl = slice(mb * MB, (mb + 1) * MB)
                    # row-paired S matmul: even m-blocks use PE rows 0-63,
                    # odd ones rows 64-127 (concurrent via tile_position)
                    half = slice(0, HID) if mb % 2 == 0 else slice(HID, P)
                    stp = psum.tile([P, NT], f32, tag="st", bufs=3, name=f"stp_{nt}_{mb}")
                    nc.tensor.matmul(
                        stp[:],
                        lhsT=k_sb[half, msl],
                        rhs=q_sb[half, ntsl],
                        start=True,
                        stop=True,
                    )
                    ex = work.tile([P, NT], bf16, tag="expst", name=f"ex_{nt}_{mb}")
                    nc.scalar.activation(ex[:], stp[:], AF.Exp)
                    first, last = (mb == 0), (mb == N_MB - 1)
                    nc.tensor.matmul(
                        y0[:], lhsT=x2t_sb[:, mb, 0:P], rhs=ex[:], start=first, stop=last
                    )
                    nc.tensor.matmul(
                        y1[:], lhsT=x2t_sb[:, mb, P:C], rhs=ex[:], start=first, stop=last
                    )
                    if mb == 3 and pending_a[0] is not None:
                        grd_prev = pending_a[0]()
                        pending_a[0] = None
                        pb = pending_b[0]
                        pending_b[0] = lambda grd_prev=grd_prev, pb=pb: pb(grd_prev)
                    if mb == 12 and pending_b[0] is not None:
                        pending_b[0]()
                        pending_b[0] = None
                    # denominator: running bf16 sum on DVE; one ones-matmul
                    # per DG m-blocks accumulated into dp
                    if mb % DG == 0:
                        acc = ex
                    else:
                        s_ = work.tile(
                            [P, NT], bf16, tag=f"dacc{mb % 2}", bufs=3,
                            name=f"ds_{nt}_{mb}",
                        )
                        nc.vector.tensor_add(s_[:], acc[:], ex[:])
                        acc = s_
                    if (mb + 1) % DG == 0:
                        n_d += 1
                        a8, nd = acc, n_d
                        def _dmm(a8=a8, nd=nd, dp=dp):
                            nc.tensor.matmul(
                                dp[:], lhsT=ones_sb[:], rhs=a8[:],
                                start=(nd == 1), stop=(nd == N_DG),
                            )
                        if nd == N_DG:
                            pending_a[0] = (
                                lambda nt=nt, y0=y0, y1=y1, dp=dp, dmm=_dmm:
                                _epilogue_a(nt, y0, y1, dp, dmm)
                            )
                            pending_b[0] = (
                                lambda state, nt=nt: _epilogue_b(nt, state)
                            )
                        else:
                            _dmm()
                        acc = None
            state_last = pending_a[0]()
            pending_b[0](state_last)

    return nc


def _split_excess_waits(nc):
    """The pinned walrus build only encodes 1 sync-wait per instruction;
    newer concourse attaches more. Hoist excess waits onto same-engine NoOps
    inserted immediately before the over-limit instruction (semantically
    identical: same engine, same program position)."""
    import concourse.mybir as mybir
    import bass_rust

    ctr = 0
    for bbl in nc.m.functions[0].blocks:
        il = bbl.instructions
        i = 0
        while i < len(il):
            inst = il[i]
            si = inst.sync_info
            limit = 1
            if si is not None and len(si.on_wait) > limit:
                waits = list(si.on_wait)
                extra = waits[limit:]
                for j in range(0, len(extra), 1):
                    nop = mybir.InstNoOp(name=f"I-wsplit-{ctr}", ins=[], outs=[])
                    ctr += 1
                    nop.engine = inst.engine
                    nop.sync_info = bass_rust.SyncInfo(
                        on_wait=[extra[j]], on_update=[]
                    )
                    il.insert(i, nop)
                    i += 1
                si.on_wait = waits[:limit]
                inst.sync_info = si
            i += 1
    return ctr


def _get_program():
    if "nc" not in _CACHE:
        _CACHE["nc"] = _build_program()
    return _CACHE["nc"]


def _get_program_hw():
    """Program with the walrus sync-wait workaround applied (breaks CoreSim's
    race detector, so only applied for hardware runs)."""
    nc = _get_program()
    if not _CACHE.get("split_done"):
        _split_excess_waits(nc)
        _CACHE["split_done"] = True
    return nc


def _make_in_maps(x1, x2, Wq, bq, Wk, bk, Wv, bv, gamma):
    g = float(np.asarray(gamma).reshape(-1)[0])
    shared = {
        "wqT": np.ascontiguousarray(Wq.T, dtype=np.float32),
        "wkT": np.ascontiguousarray(Wk.T, dtype=np.float32),
        "wvT": np.ascontiguousarray(Wv.T, dtype=np.float32),
        "bq": np.asarray(bq, dtype=np.float32).reshape(HID, 1),
        "bk": np.asarray(bk, dtype=np.float32).reshape(HID, 1),
        "gbv": (g * np.asarray(bv, dtype=np.float32)).reshape(C, 1),
        "rgam": np.array([[1.0 / g if g != 0.0 else 0.0]], dtype=np.float32),
        "onesr": np.ones((1, 128), dtype=np.float32),
    }
    in_maps = []
    for d in range(2):
        src_q, src_kv = (x1, x2) if d == 0 else (x2, x1)
        for b in range(B):
            xkv_f32 = np.ascontiguousarray(src_kv[b].reshape(C, N), dtype=np.float32)
            in_maps.append(
                {
                    "xq": np.ascontiguousarray(src_q[b].reshape(C, N), dtype=np.float32),
                    "xkv": xkv_f32,
                    "xkvb": xkv_f32.astype(ml_dtypes.bfloat16),
                    **shared,
                }
            )
    return in_maps


def kernel(x1, x2, Wq, bq, Wk, bk, Wv, bv, gamma, _want_results=False):
    x1 = np.asarray(x1, dtype=np.float32)
    x2 = np.asarray(x2, dtype=np.float32)
    nc = _get_program_hw()
    in_maps = _make_in_maps(x1, x2, Wq, bq, Wk, bk, Wv, bv, gamma)

    from concourse.bass_utils import run_bass_kernel_spmd

    res = run_bass_kernel_spmd(nc, in_maps, core_ids=list(range(2 * B)))
    outs = [r["out"].reshape(C, 64, 64) for r in res.results]
    out1 = np.stack(outs[:B]).astype(np.float32)
    out2 = np.stack(outs[B:]).astype(np.float32)
    if _want_results:
        return (out1, out2), res
    return (out1, out2)


# revision 36
# speedup vs baseline: 1.2153x; 1.2153x over previous
"""Trainium2 Bass kernel for nn_CrossModalAttention.

Problem: bidirectional cross-attention between two (B, C, H, W) feature maps.
  B=4, C=256, H=W=64 -> N=4096 pixels, HID=64.
  For each direction:  q = Wq@xq, k = Wk@xkv, v = Wv@xkv (1x1 convs),
  attn = softmax_m(q^T k), out = xq + gamma * (v @ attn^T).

Sharding: 2 directions x 4 batches = 8 independent units, one per NeuronCore.

Per-core kernel layout trick: compute S^T tiles [m(part)=128, n(free)=512] via
matmul(lhsT=k_tile, rhs=q_tile) (contraction over HID=64 on partitions), exp on
ScalarE (logits are bounded ~ +-56, so exp in f32 needs no max-subtraction),
then accumulate U[c, n] = sum_m vT[m, c]^T expS^T[m, n] directly in PSUM across
the 32 m-blocks -- no transposes anywhere. Denominator d[n] = sum_m expS^T via
a ones[128,1] matmul accumulated in PSUM. Final: out = xq + (gamma/d)*U + gamma*bv.

Precision: S path (projections + S matmul) in float32r (TF32-like; moving dim
512 >= 256 runs at full PE rate), expS^T/vT in bf16 (measured to contribute
negligible error), all accumulation in f32 PSUM.
"""

import sys

if "/opt/trn_rl_repo" not in sys.path:
    sys.path.insert(0, "/opt/trn_rl_repo")

import ml_dtypes
import numpy as np

B = 4
C = 256
HID = 64
N = 4096          # H*W
P = 128           # SBUF partitions
NT = 512          # n-tile (matmul moving free dim)
N_NT = N // NT    # 8
MB = 128          # m-block (PV contraction tile)
N_MB = N // MB    # 32
CA = C // P       # 2 c-chunks / c-blocks

_CACHE = {}


def _build_program():
    import concourse.bass as bass
    import concourse.mybir as mybir
    from concourse import tile

    f32 = mybir.dt.float32
    f32r = mybir.dt.float32r
    bf16 = mybir.dt.bfloat16
    AF = mybir.ActivationFunctionType

    nc = bass.Bass("TRN2", target_bir_lowering=False, debug=False)

    xq_d = nc.dram_tensor("xq", (C, N), f32r, kind="ExternalInput")
    xkv_d = nc.dram_tensor("xkv", (C, N), f32r, kind="ExternalInput")
    wqT_d = nc.dram_tensor("wqT", (C, HID), f32r, kind="ExternalInput")
    wkT_d = nc.dram_tensor("wkT", (C, HID), f32r, kind="ExternalInput")
    wvT_d = nc.dram_tensor("wvT", (C, C), f32r, kind="ExternalInput")
    bq_d = nc.dram_tensor("bq", (HID, 1), f32, kind="ExternalInput")
    bk_d = nc.dram_tensor("bk", (HID, 1), f32, kind="ExternalInput")
    gbv_d = nc.dram_tensor("gbv", (C, 1), f32, kind="ExternalInput")      # gamma * bv
    rgam_d = nc.dram_tensor("rgam", (1, 1), f32, kind="ExternalInput")    # 1 / gamma
    onesr_d = nc.dram_tensor("onesr", (1, P), f32r, kind="ExternalInput")  # all-ones
    x2tb_d = nc.dram_tensor("x2tb", (N, C), bf16, kind="ExternalInput")   # bf16 xkv^T
    out_d = nc.dram_tensor("out", (C, N), f32, kind="ExternalOutput")

    # c = a*128 + p views
    xq_r = xq_d[:].rearrange("(a p) n -> p a n", p=P)
    xkv_r = xkv_d[:].rearrange("(a p) n -> p a n", p=P)
    wqT_r = wqT_d[:].rearrange("(a p) h -> p a h", p=P)
    wkT_r = wkT_d[:].rearrange("(a p) h -> p a h", p=P)
    wvT_r = wvT_d[:].rearrange("(a p) c -> p a c", p=P)
    gbv_r = gbv_d[:].rearrange("(a p) one -> p (a one)", p=P)
    out_r = out_d[:].rearrange("(a p) n -> p a n", p=P)

    with tile.TileContext(nc) as tc:
        with (
            tc.tile_pool(name="const", bufs=1) as const,
            tc.tile_pool(name="xin", bufs=1) as xin,
            tc.tile_pool(name="qk", bufs=1) as qk,
            tc.tile_pool(name="vtp", bufs=1) as vtp,
            tc.tile_pool(name="work", bufs=3) as work,
            tc.tile_pool(name="ep", bufs=2) as ep,
            tc.tile_pool(name="dram", bufs=2, space="DRAM") as dram,
            tc.tile_pool(name="psum", bufs=1, space="PSUM") as psum,
        ):
            # ---- constants / weights ----
            wq_sb = const.tile([P, CA, HID], f32r, tag="wq")
            nc.sync.dma_start(wq_sb[:], wqT_r)
            wk_sb = const.tile([P, CA, HID], f32r, tag="wk")
            nc.sync.dma_start(wk_sb[:], wkT_r)
            wv_sb = const.tile([P, CA, C], f32r, tag="wv")
            nc.sync.dma_start(wv_sb[:], wvT_r)
            bq_sb = const.tile([HID, 1], f32, tag="bq")
            nc.sync.dma_start(bq_sb[:], bq_d[:])
            bk_sb = const.tile([HID, 1], f32, tag="bk")
            nc.sync.dma_start(bk_sb[:], bk_d[:])
            gbv_sb = const.tile([P, CA], f32, tag="gbv")
            nc.sync.dma_start(gbv_sb[:], gbv_r)
            rgam_sb = const.tile([1, 1], f32, tag="rgam")
            nc.sync.dma_start(rgam_sb[:], rgam_d[:])
            ones_sb = const.tile([P, 1], bf16, tag="ones")
            nc.vector.memset(ones_sb[:], 1.0)
            onesr_sb = const.tile([1, P], f32r, tag="onesr")
            nc.sync.dma_start(onesr_sb[:], onesr_d[:])

            # ---- x loads (chunked for DMA/compute overlap) ----
            xq_sb = xin.tile([P, CA, N], f32r, tag="xq")
            xkv_sb = xin.tile([P, CA, N], f32r, tag="xkv")
            NCH = 1024
            for h in range(N // NCH):
                sl = slice(h * NCH, (h + 1) * NCH)
                for a in range(CA):
                    nc.sync.dma_start(xkv_sb[:, a, sl], xkv_r[:, a, sl])
            for h in range(N // NCH):
                sl = slice(h * NCH, (h + 1) * NCH)
                for a in range(CA):
                    nc.sync.dma_start(xq_sb[:, a, sl], xq_r[:, a, sl])

            # ---- projections ----
            # q/k stored twice (rows 0-63 and 64-127) so the K=64 S-matmuls
            # can be row-paired into both halves of the PE array.
            q_sb = qk.tile([P, N], f32r, tag="q")
            k_sb = qk.tile([P, N], f32r, tag="k")
            # k projection first: it only needs xkv, which is DMA'd first
            for nt in range(N_NT):
                ntsl = slice(nt * NT, (nt + 1) * NT)
                kp = psum.tile([P, NT], f32, tag="st", bufs=3)
                for a in range(CA):
                    nc.tensor.matmul(
                        kp[:HID, :],
                        lhsT=wk_sb[:, a, :],
                        rhs=xkv_sb[:, a, ntsl],
                        start=(a == 0),
                        stop=(a == CA - 1),
                    )
                nc.vector.tensor_scalar_add(k_sb[0:HID, ntsl], kp[:HID, :], bk_sb[:])
                nc.vector.tensor_scalar_add(k_sb[HID:P, ntsl], kp[:HID, :], bk_sb[:])

            # X2^T tiles [m, c_in] (bf16) for the Y = X2 @ E matmuls; the
            # transpose+cast happens on the host. The v projection is folded
            # in AFTER the attention sum via associativity:
            # U = Wv @ (X2 @ E) -- saves the whole vT projection pass.
            x2t_sb = vtp.tile([P, N_MB, C], bf16, tag="x2t")
            x2t_r = x2tb_d[:].rearrange("(mb p) c -> p mb c", p=P)
            for mb in range(N_MB):
                nc.sync.dma_start(x2t_sb[:, mb, :], x2t_r[:, mb, :])

            # q projection (needs xq, which lands after xkv)
            for nt in range(N_NT):
                ntsl = slice(nt * NT, (nt + 1) * NT)
                qp = psum.tile([P, NT], f32, tag="st", bufs=3)
                for a in range(CA):
                    nc.tensor.matmul(
                        qp[:HID, :],
                        lhsT=wq_sb[:, a, :],
                        rhs=xq_sb[:, a, ntsl],
                        start=(a == 0),
                        stop=(a == CA - 1),
                    )
                nc.vector.tensor_scalar_add(q_sb[0:HID, ntsl], qp[:HID, :], bq_sb[:])
                nc.vector.tensor_scalar_add(q_sb[HID:P, ntsl], qp[:HID, :], bq_sb[:])

            # ---- attention ----
            DG = 8           # m-blocks per denominator group
            N_DG = N_MB // DG

            def _epilogue_a(nt, y0, y1, dp, final_dmm):
                # finish d, grd = gamma/d on ACT+DVE, kick off the partition-
                # broadcast DRAM roundtrip, and move Y to SBUF (no PE work)
                final_dmm()
                rd = ep.tile([1, NT], f32, tag="rd", name=f"rd_{nt}")
                nc.scalar.activation(rd[:], dp[:], AF.Copy, scale=rgam_sb[:])
                grd = ep.tile([1, NT], f32, tag="grd", name=f"grd_{nt}")
                nc.vector.reciprocal(grd[:], rd[:])
                dscr = dram.tile([1, NT], f32, tag="dscr", name=f"dscr_{nt}")
                nc.sync.dma_start(dscr[:], grd[:])
                rdb = ep.tile([P, NT], f32, tag="rdb", name=f"rdb_{nt}")
                nc.sync.dma_start(rdb[:], dscr[:].broadcast_to((P, NT)))
                yb0 = ep.tile([P, NT], f32r, tag="yb0", name=f"yb0_{nt}")
                nc.scalar.copy(yb0[:], y0[:])
                yb1 = ep.tile([P, NT], f32r, tag="yb1", name=f"yb1_{nt}")
                nc.scalar.copy(yb1[:], y1[:])
                return rdb, yb0, yb1

            def _epilogue_b(nt, state):
                # U = Wv @ Y (f32r, 2 accumulating matmuls per c-block), then
                # out[c, n] = xq + rdb[n] * U[c, n] + gamma*bv[c]
                rdb, yb0, yb1 = state
                ntsl = slice(nt * NT, (nt + 1) * NT)
                for cb in range(CA):
                    ups = psum.tile(
                        [P, NT], f32, tag="st", bufs=3, name=f"ups_{nt}_{cb}"
                    )
                    nc.tensor.matmul(
                        ups[:], lhsT=wv_sb[:, 0, cb * P : (cb + 1) * P],
                        rhs=yb0[:], start=True, stop=False,
                    )
                    nc.tensor.matmul(
                        ups[:], lhsT=wv_sb[:, 1, cb * P : (cb + 1) * P],
                        rhs=yb1[:], start=False, stop=True,
                    )
                    t = ep.tile([P, NT], f32, tag="t", name=f"t_{nt}_{cb}")
                    nc.vector.tensor_mul(t[:], ups[:], rdb[:])
                    o = ep.tile([P, NT], f32, tag="o", name=f"o_{nt}_{cb}")
                    nc.vector.scalar_tensor_tensor(
                        o[:],
                        in0=t[:],
                        scalar=gbv_sb[:, cb : cb + 1],
                        in1=xq_sb[:, cb, ntsl],
                        op0=mybir.AluOpType.add,
                        op1=mybir.AluOpType.add,
                    )
                    nc.sync.dma_start(out_r[:, cb, ntsl], o[:])

            # previous n-tile's epilogue stages, deferred into the next
            # n-tile's m-loop so they never stall the in-order PE queue
            pending_a = [None]
            pending_b = [None]

            for nt in range(N_NT):
                ntsl = slice(nt * NT, (nt + 1) * NT)
                y0 = psum.tile([P, NT], f32, tag="y", bufs=4, name=f"y0_{nt}")
                y1 = psum.tile([P, NT], f32, tag="y", bufs=4, name=f"y1_{nt}")
                dp = psum.tile([1, NT], f32, tag="dd", bufs=1, name=f"dp_{nt}")
                acc = None   # running bf16 partial-sum for the current d group
                n_d = 0      # d-matmuls issued for this n-tile
                for mb in range(N_MB):
                    msl = slice(mb * MB, (mb + 1) * MB)
                    # row-paired S matmul: even m-blocks use PE rows 0-63,
                    # odd ones rows 64-127 (concurrent via tile_position)
                    half = slice(0, HID) if mb % 2 == 0 else slice(HID, P)
                    stp = psum.tile([P, NT], f32, tag="st", bufs=3, name=f"stp_{nt}_{mb}")
                    nc.tensor.matmul(
                        stp[:],
                        lhsT=k_sb[half, msl],
                        rhs=q_sb[half, ntsl],
                        start=True,
                        stop=True,
                    )
                    ex = work.tile([P, NT], bf16, tag="expst", name=f"ex_{nt}_{mb}")
                    nc.scalar.activation(ex[:], stp[:], AF.Exp)
                    first, last = (mb == 0), (mb == N_MB - 1)
                    nc.tensor.matmul(
                        y0[:], lhsT=x2t_sb[:, mb, 0:P], rhs=ex[:], start=first, stop=last
                    )
                    nc.tensor.matmul(
                        y1[:], lhsT=x2t_sb[:, mb, P:C], rhs=ex[:], start=first, stop=last
                    )
                    if mb == 3 and pending_a[0] is not None:
                        grd_prev = pending_a[0]()
                        pending_a[0] = None
                        pb = pending_b[0]
                        pending_b[0] = lambda grd_prev=grd_prev, pb=pb: pb(grd_prev)
                    if mb == 12 and pending_b[0] is not None:
                        pending_b[0]()
                        pending_b[0] = None
                    # denominator: running bf16 sum on DVE; one ones-matmul
                    # per DG m-blocks accumulated into dp
                    if mb % DG == 0:
                        acc = ex
                    else:
                        s_ = work.tile(
                            [P, NT], bf16, tag=f"dacc{mb % 2}", bufs=3,
                            name=f"ds_{nt}_{mb}",
                        )
                        nc.vector.tensor_add(s_[:], acc[:], ex[:])
                        acc = s_
                    if (mb + 1) % DG == 0:
                        n_d += 1
                        a8, nd = acc, n_d
                        def _dmm(a8=a8, nd=nd, dp=dp):
                            nc.tensor.matmul(
                                dp[:], lhsT=ones_sb[:], rhs=a8[:],
                                start=(nd == 1), stop=(nd == N_DG),
                            )
                        if nd == N_DG:
                            pending_a[0] = (
                                lambda nt=nt, y0=y0, y1=y1, dp=dp, dmm=_dmm:
                                _epilogue_a(nt, y0, y1, dp, dmm)
                            )
                            pending_b[0] = (
                                lambda state, nt=nt: _epilogue_b(nt, state)
                            )
                        else:
                            _dmm()
                        acc = None
            state_last = pending_a[0]()
            pending_b[0](state_last)

    return nc


def _split_excess_waits(nc):
    """The pinned walrus build only encodes 1 sync-wait per instruction;
    newer concourse attaches more. Hoist excess waits onto same-engine NoOps
    inserted immediately before the over-limit instruction (semantically
    identical: same engine, same program position)."""
    import concourse.mybir as mybir
    import bass_rust

    ctr = 0
    for bbl in nc.m.functions[0].blocks:
        il = bbl.instructions
        i = 0
        while i < len(il):
            inst = il[i]
            si = inst.sync_info
            limit = 1
            if si is not None and len(si.on_wait) > limit:
                waits = list(si.on_wait)
                extra = waits[limit:]
                for j in range(0, len(extra), 1):
                    nop = mybir.InstNoOp(name=f"I-wsplit-{ctr}", ins=[], outs=[])
                    ctr += 1
                    nop.engine = inst.engine
                    nop.sync_info = bass_rust.SyncInfo(
                        on_wait=[extra[j]], on_update=[]
                    )
                    il.insert(i, nop)
                    i += 1
                si.on_wait = waits[:limit]
                inst.sync_info = si
            i += 1
    return ctr


def _get_program():
    if "nc" not in _CACHE:
        _CACHE["nc"] = _build_program()
    return _CACHE["nc"]


def _get_program_hw():
    """Program with the walrus sync-wait workaround applied (breaks CoreSim's
    race detector, so only applied for hardware runs)."""
    nc = _get_program()
    if not _CACHE.get("split_done"):
        _split_excess_waits(nc)
        _CACHE["split_done"] = True
    return nc


def _make_in_maps(x1, x2, Wq, bq, Wk, bk, Wv, bv, gamma):
    g = float(np.asarray(gamma).reshape(-1)[0])
    shared = {
        "wqT": np.ascontiguousarray(Wq.T, dtype=np.float32),
        "wkT": np.ascontiguousarray(Wk.T, dtype=np.float32),
        "wvT": np.ascontiguousarray(Wv.T, dtype=np.float32),
        "bq": np.asarray(bq, dtype=np.float32).reshape(HID, 1),
        "bk": np.asarray(bk, dtype=np.float32).reshape(HID, 1),
        "gbv": (g * np.asarray(bv, dtype=np.float32)).reshape(C, 1),
        "rgam": np.array([[1.0 / g if g != 0.0 else 0.0]], dtype=np.float32),
        "onesr": np.ones((1, 128), dtype=np.float32),
    }
    in_maps = []
    for d in range(2):
        src_q, src_kv = (x1, x2) if d == 0 else (x2, x1)
        for b in range(B):
            xkv_f32 = np.ascontiguousarray(src_kv[b].reshape(C, N), dtype=np.float32)
            in_maps.append(
                {
                    "xq": np.ascontiguousarray(src_q[b].reshape(C, N), dtype=np.float32),
                    "xkv": xkv_f32,
                    "x2tb": np.ascontiguousarray(xkv_f32.T).astype(ml_dtypes.bfloat16),
                    **shared,
                }
            )
    return in_maps


def kernel(x1, x2, Wq, bq, Wk, bk, Wv, bv, gamma, _want_results=False):
    x1 = np.asarray(x1, dtype=np.float32)
    x2 = np.asarray(x2, dtype=np.float32)
    nc = _get_program_hw()
    in_maps = _make_in_maps(x1, x2, Wq, bq, Wk, bk, Wv, bv, gamma)

    from concourse.bass_utils import run_bass_kernel_spmd

    res = run_bass_kernel_spmd(nc, in_maps, core_ids=list(range(2 * B)))
    outs = [r["out"].reshape(C, 64, 64) for r in res.results]
    out1 = np.stack(outs[:B]).astype(np.float32)
    out2 = np.stack(outs[B:]).astype(np.float32)
    if _want_results:
        return (out1, out2), res
    return (out1, out2)
